# revision 16
# baseline (speedup 1.0000x reference)
import sys
sys.path.insert(0, '/opt/trn_rl_repo')
import numpy as np

import concourse.bacc as bacc
import concourse.tile as tile
import concourse.mybir as mybir
from concourse.bass_utils import run_bass_kernel_spmd

F32 = mybir.dt.float32
F32R = mybir.dt.float32r
F16 = mybir.dt.float16
ALU = mybir.AluOpType
ACTF = mybir.ActivationFunctionType

# problem dims (hardcoded)
B, CIN, COUT, L, COND = 32, 256, 512, 256, 512
DSTATE, DCONV = 16, 4
DIN = 1024
DTR = 32
MLPH = 2048
NG = 8
K = 3
NCORES = 8
NB = B // NCORES          # 4 batches per core
T = NB * L                # 1024 tokens per core
P = 128
CI = CIN // P             # 2 input-channel chunks
CO = COUT // P            # 4 out-channel chunks
CD = DIN // P             # 8 d_inner chunks
CM = MLPH // P            # 16 mlp chunks
GSZ = (COUT // NG) * L    # group-norm group size = 64*256
NBLK = 4                  # n-state blocks for B/C broadcast
NSB = DSTATE // NBLK      # 8 states per block


def _chunks(w):
    kt = w.shape[0]
    return np.ascontiguousarray(
        w.reshape(kt // P, P, *w.shape[1:]).transpose(1, 0, *range(2, w.ndim + 1)))


def _cols(v):
    return np.ascontiguousarray(v.reshape(-1, P).T)


def build_program():
    nc = bacc.Bacc(trn_type="TRN2")

    def din(name, shape, dt):
        return nc.dram_tensor(name, list(shape), dt, kind="ExternalInput")

    x_pad = din("x_pad", [P, CI, NB, L + K], F32R)
    cond_cf = din("cond_cf", [P, COND // P, NB], F32)
    conv_wT = din("conv_wT", [P, CI, K, COUT], F32R)
    res_wT = din("res_wT", [P, CI, COUT], F32R)
    gn_w = din("gn_w", [P, CO], F32)
    gn_wneg = din("gn_wneg", [P, CO], F32)
    gn_b = din("gn_b", [P, CO], F32)
    conv_b = din("conv_b", [P, CO], F32)
    res_b = din("res_b", [P, CO], F32)
    film_w16 = din("film_w16", [P, COND // P, 2 * COUT], F16)
    film_b = din("film_b", [P, 2 * COUT // P], F32)
    win16 = din("win16", [P, CO, 2 * DIN], F16)
    bin_c = din("bin_c", [P, 2 * DIN // P], F32)
    xproj16 = din("xproj16", [P, CD, DTR + 2 * DSTATE], F16)
    dtproj16 = din("dtproj16", [DTR, DIN], F16)
    dtb_neg = din("dtb_neg", [P, CD], F32)
    negA = din("negA", [P, CD, DSTATE], F32)
    dconv_wc = din("dconv_wc", [P, CD, DCONV], F32)
    dconv_bc = din("dconv_bc", [P, CD], F32)
    dskip_c = din("dskip_c", [P, CD], F32)
    wout16 = din("wout16", [P, CD, COUT], F16)
    wfc1_16 = din("wfc1_16", [P, CO, MLPH], F16)
    bfc1_c = din("bfc1_c", [P, CM], F32)
    wfc2_16 = din("wfc2_16", [P, CM, COUT], F16)
    fc2_bc = din("fc2_bc", [P, CO], F32)
    gmask = din("gmask", [P, 2], F16)
    gexpand = din("gexpand", [2, P], F32)
    ones_c = din("ones_c", [P, 1], F16)

    out_d = nc.dram_tensor("out", [P, CO, NB, L], F32, kind="ExternalOutput")

    with tile.TileContext(nc) as tc, nc.allow_low_precision("fp16 state by design"):
        with tc.tile_pool(name="wp", bufs=2) as wp, \
             tc.tile_pool(name="cst", bufs=1) as cst, \
             tc.tile_pool(name="act", bufs=1) as ap, \
             tc.tile_pool(name="p16", bufs=2) as p16, \
             tc.tile_pool(name="b16", bufs=3) as b16, \
             tc.tile_pool(name="row", bufs=4) as rowp, \
             tc.tile_pool(name="sl", bufs=2) as sl, \
             tc.tile_pool(name="sl6", bufs=8) as sl6, \
             tc.tile_pool(name="sl1", bufs=1) as sl1, \
             tc.tile_pool(name="sc", bufs=3) as scp, \
             tc.tile_pool(name="dram", bufs=1, space="DRAM") as dr, \
             tc.tile_pool(name="mm", bufs=2, space="PSUM") as pmm, \
             tc.tile_pool(name="pst", bufs=2, space="PSUM") as pst, \
             tc.tile_pool(name="opj", bufs=1, space="PSUM") as popj:

            def load(pool, dram, tag=None, name=None):
                t = pool.tile(list(dram.shape), dram.dtype, tag=tag,
                              name=name or "t_" + dram.name)
                nc.sync.dma_start(t[:], dram[:])
                return t

            t_gnw = load(cst, gn_w)
            t_gnwn = load(cst, gn_wneg)
            t_gnb = load(cst, gn_b)
            t_convb = load(cst, conv_b)
            t_resb = load(cst, res_b)
            t_filmb = load(cst, film_b)
            t_binc = load(cst, bin_c)
            t_xproj = load(cst, xproj16)
            t_dtproj = load(cst, dtproj16)
            t_dtbn = load(cst, dtb_neg)
            t_negA = load(cst, negA)
            t_dconvw = load(cst, dconv_wc)
            t_dconvb = load(cst, dconv_bc)
            t_dskip = load(cst, dskip_c)
            t_bfc1 = load(cst, bfc1_c)
            t_fc2b = load(cst, fc2_bc)
            t_gmask = load(cst, gmask)
            t_gexp = load(cst, gexpand)
            t_ones = load(cst, ones_c)
            t_cond = load(cst, cond_cf)
            eps2 = cst.tile([2, 1], F32)
            nc.vector.memset(eps2[:], 1e-5)
            eps1 = cst.tile([1, 1], F32)
            nc.vector.memset(eps1[:], 1e-5)

            # ============ P1: conv(k=3) ============
            t_xpad = load(wp, x_pad, tag="bigw", name="xpad1")
            t_convw = load(wp, conv_wT, tag="bigw", name="convw")
            conv_out = p16.tile([P, CO, NB, L], F16, tag="p16", name="convout")
            for oc in range(CO):
                for bp in range(NB // 2):
                    pt = pmm.tile([P, 2, L], F32, tag="mm", name="ptc")
                    first = True
                    for k in range(K):
                        for ic in range(CI):
                            nc.tensor.matmul(
                                pt[:], t_convw[:, ic, k, oc * P:(oc + 1) * P],
                                t_xpad[:, ic, 2 * bp:2 * bp + 2, k:k + L],
                                start=first, stop=(k == K - 1 and ic == CI - 1))
                            first = False
                    nc.scalar.activation(
                        conv_out[:, oc, 2 * bp:2 * bp + 2, :], pt[:],
                        ACTF.Identity, bias=t_convb[:, oc:oc + 1], scale=1.0)

            # ============ P2: GroupNorm stats ============
            s1 = ap.tile([2, CO, NB, 1], F32, tag="s1")
            s2 = ap.tile([2, CO, NB, 1], F32, tag="s2")
            for oc in range(CO):
                sq = sl.tile([P, NB, L], F16, tag="sq")
                nc.scalar.activation(sq[:], conv_out[:, oc], ACTF.Square,
                                     bias=0.0, scale=1.0)
                for h in range(2):
                    p1 = pmm.tile([2, 2, L], F32, tag="mm", name="pg1")
                    nc.tensor.matmul(p1.rearrange("p a b -> p (a b)"), t_gmask[:],
                                     conv_out[:, oc, 2 * h:2 * h + 2, :].rearrange(
                                         "p a b -> p (a b)"),
                                     start=True, stop=True)
                    nc.vector.tensor_reduce(s1[:, oc, 2 * h:2 * h + 2, :], p1[:],
                                            axis=mybir.AxisListType.X, op=ALU.add)
                    p2 = pmm.tile([2, 2, L], F32, tag="mm", name="pg2")
                    nc.tensor.matmul(p2.rearrange("p a b -> p (a b)"), t_gmask[:],
                                     sq[:, 2 * h:2 * h + 2, :].rearrange(
                                         "p a b -> p (a b)"),
                                     start=True, stop=True)
                    nc.vector.tensor_reduce(s2[:, oc, 2 * h:2 * h + 2, :], p2[:],
                                            axis=mybir.AxisListType.X, op=ALU.add)
            mean_g = ap.tile([2, CO, NB, 1], F32, tag="meang")
            nc.vector.tensor_scalar_mul(mean_g[:], s1[:], 1.0 / GSZ)
            msq = ap.tile([2, CO, NB, 1], F32, tag="msq")
            nc.vector.tensor_mul(msq[:], mean_g[:], mean_g[:])
            var_g = ap.tile([2, CO, NB, 1], F32, tag="varg")
            nc.vector.scalar_tensor_tensor(var_g[:], s2[:], 1.0 / GSZ, msq[:],
                                           op0=ALU.mult, op1=ALU.subtract)
            rstd_g = ap.tile([2, CO, NB, 1], F32, tag="rstdg")
            nc.scalar.activation(rstd_g[:], var_g[:], ACTF.Ln, bias=eps2[:], scale=1.0)
            nc.scalar.activation(rstd_g[:], rstd_g[:], ACTF.Exp, bias=0.0, scale=-0.5)
            mr_g = ap.tile([2, CO, NB, 1], F32, tag="mrg")
            nc.vector.tensor_mul(mr_g[:], mean_g[:], rstd_g[:])
            exp_in = ap.tile([2, CO, 2, NB], F32, tag="expin")
            nc.vector.tensor_copy(exp_in[:, :, 0, :], rstd_g[:, :, :, 0])
            nc.vector.tensor_copy(exp_in[:, :, 1, :], mr_g[:, :, :, 0])
            exp_t = ap.tile([P, CO, 2, NB], F32, tag="expt")
            for oc in range(CO):
                pe = pmm.tile([P, 2 * NB], F32, tag="mm", name="pge")
                nc.tensor.matmul(pe[:], t_gexp[:],
                                 exp_in[:, oc].rearrange("p a b -> p (a b)"),
                                 start=True, stop=True)
                nc.scalar.copy(exp_t[:, oc], pe.rearrange("p (a b) -> p a b", a=2))
            gnA = ap.tile([P, CO, NB], F32, tag="gnA")
            gnB = ap.tile([P, CO, NB], F32, tag="gnB")
            for oc in range(CO):
                nc.vector.tensor_scalar_mul(gnA[:, oc], exp_t[:, oc, 0], t_gnw[:, oc:oc + 1])
                nc.vector.scalar_tensor_tensor(
                    gnB[:, oc], exp_t[:, oc, 1], t_gnwn[:, oc:oc + 1],
                    t_gnb[:, oc:oc + 1].to_broadcast([P, NB]),
                    op0=ALU.mult, op1=ALU.add)

            # ============ P3: GN apply + mish + FiLM ============
            v_t = ap.tile([P, CO, NB, L], F16, tag="vt")
            for oc in range(CO):
                for b in range(NB):
                    nc.vector.scalar_tensor_tensor(
                        v_t[:, oc, b], conv_out[:, oc, b], gnA[:, oc, b:b + 1],
                        gnB[:, oc, b:b + 1].to_broadcast([P, L]),
                        op0=ALU.mult, op1=ALU.add)
            msh = [rowp.tile([P, NB, L], F16, tag="row", name=f"msh{oc}")
                   for oc in range(CO)]
            csg = ap.tile([P, COND // P, NB], F32, tag="csg")
            for oc in range(CO):
                nc.scalar.activation(msh[oc][:], v_t[:, oc], ACTF.Sigmoid,
                                     bias=0.0, scale=-1.0)
            nc.scalar.activation(csg[:], t_cond[:], ACTF.Sigmoid, bias=0.0, scale=-1.0)
            for oc in range(CO):
                nc.scalar.activation(msh[oc][:], msh[oc][:], ACTF.Ln, bias=0.0, scale=1.0)
            nc.scalar.activation(csg[:], csg[:], ACTF.Ln, bias=0.0, scale=1.0)
            for oc in range(CO):
                nc.scalar.activation(msh[oc][:], msh[oc][:], ACTF.Tanh, bias=0.0, scale=-1.0)
            nc.scalar.activation(csg[:], csg[:], ACTF.Tanh, bias=0.0, scale=-1.0)
            for oc in range(CO):
                nc.vector.tensor_mul(v_t[:, oc], v_t[:, oc], msh[oc][:])
            mish_c = ap.tile([P, COND // P, NB], F16, tag="mishc")
            nc.vector.tensor_mul(mish_c[:], t_cond[:], csg[:])
            t_filmw = load(wp, film_w16, tag="bigw", name="filmw")
            film_sc = ap.tile([P, 2 * COUT // P, NB], F32, tag="filmsc")
            for mc in range(2 * COUT // P):
                pf = pmm.tile([P, NB], F32, tag="mm", name="pfm")
                for kc in range(COND // P):
                    nc.tensor.matmul(pf[:], t_filmw[:, kc, mc * P:(mc + 1) * P],
                                     mish_c[:, kc], start=(kc == 0),
                                     stop=(kc == COND // P - 1))
                nc.scalar.activation(film_sc[:, mc], pf[:], ACTF.Identity,
                                     bias=t_filmb[:, mc:mc + 1], scale=1.0)
            out_t = ap.tile([P, CO, NB, L], F16, tag="outt")
            for oc in range(CO):
                for b in range(NB):
                    nc.vector.scalar_tensor_tensor(
                        out_t[:, oc, b], v_t[:, oc, b], film_sc[:, oc, b:b + 1],
                        film_sc[:, CO + oc, b:b + 1].to_broadcast([P, L]),
                        op0=ALU.mult, op1=ALU.add)

            # ============ layernorm helper (src f16 [P,CO,NB,L] -> dst f16 [P,CO,T]) ====
            def layernorm_16(src, xh, tag):
                stt1 = pst.tile([33, 512], F32, tag="st", name="st1" + tag)
                stt2 = pst.tile([33, 512], F32, tag="st", name="st2" + tag)
                ps1 = [stt1[0:1, :], stt1[32:33, :]]
                ps2 = [stt2[0:1, :], stt2[32:33, :]]
                for oc in range(CO):
                    sq = sl.tile([P, NB, L], F16, tag="sq")
                    nc.scalar.activation(sq[:], src[:, oc], ACTF.Square, bias=0.0, scale=1.0)
                    for h in range(2):
                        nc.tensor.matmul(ps1[h][:], t_ones[:],
                                         src[:, oc, 2 * h:2 * h + 2, :].rearrange(
                                             "p a b -> p (a b)"),
                                         start=(oc == 0), stop=(oc == CO - 1))
                        nc.tensor.matmul(ps2[h][:], t_ones[:],
                                         sq[:, 2 * h:2 * h + 2, :].rearrange(
                                             "p a b -> p (a b)"),
                                         start=(oc == 0), stop=(oc == CO - 1))
                s1r = rowp.tile([1, T], F32, tag="row", name="s1r" + tag)
                s2r = rowp.tile([1, T], F32, tag="row", name="s2r" + tag)
                for h in range(2):
                    nc.scalar.copy(s1r[:, 512 * h:512 * (h + 1)], ps1[h][:])
                    nc.scalar.copy(s2r[:, 512 * h:512 * (h + 1)], ps2[h][:])
                nc.vector.tensor_scalar_mul(s1r[:], s1r[:], 1.0 / COUT)   # mu
                musq = rowp.tile([1, T], F32, tag="row", name="musq" + tag)
                nc.vector.tensor_mul(musq[:], s1r[:], s1r[:])
                nc.vector.scalar_tensor_tensor(s2r[:], s2r[:], 1.0 / COUT, musq[:],
                                               op0=ALU.mult, op1=ALU.subtract)  # var
                nc.scalar.activation(s2r[:], s2r[:], ACTF.Ln, bias=eps1[:], scale=1.0)
                nc.scalar.activation(s2r[:], s2r[:], ACTF.Exp, bias=0.0, scale=-0.5)  # rs
                cc = rowp.tile([1, T], F32, tag="row", name="cc" + tag)
                nc.vector.scalar_tensor_tensor(cc[:], s1r[:], -1.0, s2r[:],
                                               op0=ALU.mult, op1=ALU.mult)  # -mu*rs
                rs16 = rowp.tile([1, T], F16, tag="row", name="rs16" + tag)
                nc.vector.tensor_copy(rs16[:], s2r[:])
                cc16 = rowp.tile([1, T], F16, tag="row", name="cc16" + tag)
                nc.vector.tensor_copy(cc16[:], cc[:])
                rs_b = sl1.tile([P, T], F16, tag="lnrsb")
                cc_b = sl1.tile([P, T], F16, tag="lnccb")
                nc.gpsimd.partition_broadcast(rs_b[:], rs16[:])
                nc.gpsimd.partition_broadcast(cc_b[:], cc16[:])
                for oc in range(CO):
                    tmp = sl1.tile([P, NB, L], F16, tag="lntmp")
                    nc.vector.tensor_mul(tmp[:], src[:, oc],
                                         rs_b.rearrange("p (b l) -> p b l", b=NB))
                    nc.vector.tensor_add(xh[:, oc],
                                         tmp.rearrange("p b l -> p (b l)"), cc_b[:])

            # ============ P4: LN1 + in_proj ============
            xh16 = ap.tile([P, CO, T], F16, tag="xh16")
            layernorm_16(out_t, xh16, "a")
            t_win = load(wp, win16, tag="bigw", name="winw")
            xc_pad = p16.tile([P, CD, NB, L + K], F16, tag="p16", name="xcpad")
            nc.vector.memset(xc_pad[:], 0.0)
            z_dram = dr.tile([P, CD, T], F16, name="zdram")
            for mc in range(2 * DIN // P):
                for h in range(2):
                    pt = pmm.tile([P, 512], F32, tag="mm", name="pti")
                    for kc in range(CO):
                        nc.tensor.matmul(pt[:], t_win[:, kc, mc * P:(mc + 1) * P],
                                         xh16[:, kc, 512 * h:512 * (h + 1)],
                                         start=(kc == 0), stop=(kc == CO - 1))
                    if mc < CD:
                        nc.scalar.activation(
                            xc_pad[:, mc, 2 * h:2 * h + 2, K:K + L],
                            pt.rearrange("p (b l) -> p b l", b=2),
                            ACTF.Identity, bias=t_binc[:, mc:mc + 1], scale=1.0)
                    else:
                        zz = sl6.tile([P, 512], F16, tag="s2k", name="zz")
                        nc.scalar.activation(zz[:], pt[:], ACTF.Identity,
                                             bias=t_binc[:, mc:mc + 1], scale=1.0)
                        nc.sync.dma_start(z_dram[:, mc - CD, 512 * h:512 * (h + 1)], zz[:])

            # ============ P5: dw-conv + silu + x_proj + dt ============
            xcs16 = p16.tile([P, CD, T], F16, tag="p16", name="xcs16")
            xcs_dram = dr.tile([P, CD, T], F16, name="xcsdram")
            for mc in range(CD):
                for bp in range(NB // 2):
                    acc = sl.tile([P, 2, L], F16, tag="dwacc")
                    nc.vector.tensor_scalar_mul(
                        acc[:], xc_pad[:, mc, 2 * bp:2 * bp + 2, 0:L],
                        t_dconvw[:, mc, 0:1])
                    for k in range(1, DCONV):
                        nc.vector.scalar_tensor_tensor(
                            acc[:], xc_pad[:, mc, 2 * bp:2 * bp + 2, k:k + L],
                            t_dconvw[:, mc, k:k + 1], acc[:],
                            op0=ALU.mult, op1=ALU.add)
                    nc.scalar.activation(
                        xcs16[:, mc, 512 * bp:512 * (bp + 1)],
                        acc.rearrange("p b l -> p (b l)"),
                        ACTF.Silu, bias=t_dconvb[:, mc:mc + 1], scale=1.0)
                nc.sync.dma_start(xcs_dram[:, mc], xcs16[:, mc])
            dbl = ap.tile([DTR + 2 * DSTATE, T], F16, tag="dbl")
            for h in range(2):
                pd = pmm.tile([DTR + 2 * DSTATE, 512], F32, tag="mm", name="pdx")
                for kc in range(CD):
                    nc.tensor.matmul(pd[:], t_xproj[:, kc],
                                     xcs16[:, kc, 512 * h:512 * (h + 1)],
                                     start=(kc == 0), stop=(kc == CD - 1))
                nc.scalar.copy(dbl[:, 512 * h:512 * (h + 1)], pd[:])
            sg_dram = dr.tile([P, CD, T], F16, name="sgdram")
            for mc in range(CD):
                us = sl6.tile([P, T], F16, tag="s2k", name="us")
                for h in range(2):
                    pt = pmm.tile([P, 512], F32, tag="mm", name="ptd")
                    nc.tensor.matmul(pt[:], t_dtproj[:, mc * P:(mc + 1) * P],
                                     dbl[0:DTR, 512 * h:512 * (h + 1)],
                                     start=True, stop=True)
                    nc.scalar.activation(us[:, 512 * h:512 * (h + 1)], pt[:],
                                         ACTF.Sigmoid, bias=t_dtbn[:, mc:mc + 1], scale=-1.0)
                nc.scalar.activation(us[:], us[:], ACTF.Ln, bias=0.0, scale=1.0)
                nc.sync.dma_start(sg_dram[:, mc], us[:])

            # ============ P6: selective scan (n-blocked) ============
            t_wout = load(wp, wout16, tag="bigw", name="woutw")
            y4 = p16.tile([P, CD, T], F16, tag="p16", name="y4")
            for blk in range(NBLK):
                Bb = b16.tile([P, NSB, T], F16, tag="b16", name=f"Bb{blk}")
                Cb = b16.tile([P, NSB, T], F16, tag="b16", name=f"Cb{blk}")
                for n8 in range(NSB):
                    n = blk * NSB + n8
                    stB = sl6.tile([1, T], F16, tag="s2k", name="stB")
                    nc.sync.dma_start(stB[:], dbl[DTR + n:DTR + n + 1, :])
                    nc.gpsimd.partition_broadcast(Bb[:, n8], stB[:])
                    stC = sl6.tile([1, T], F16, tag="s2k", name="stC")
                    nc.sync.dma_start(stC[:], dbl[DTR + DSTATE + n:DTR + DSTATE + n + 1, :])
                    nc.gpsimd.partition_broadcast(Cb[:, n8], stC[:])
                for mc in range(CD):
                    u = sl6.tile([P, T], F16, tag="s2k", name="u")
                    nc.sync.dma_start(u[:], sg_dram[:, mc])
                    dtxs = sl6.tile([P, T], F16, tag="s2k", name="dtxs")
                    nc.sync.dma_start(dtxs[:], xcs_dram[:, mc])
                    dtx = sl6.tile([P, T], F16, tag="s2k", name="dtx")
                    nc.vector.scalar_tensor_tensor(dtx[:], u[:], -1.0, dtxs[:],
                                                   op0=ALU.mult, op1=ALU.mult)
                    pr = sl1.tile([P, NSB // 2, T], F16, tag="pr")
                    ch_prev = None
                    for n8 in range(NSB):
                        n = blk * NSB + n8
                        dA = scp.tile([P, T], F16, tag="dA")
                        nc.scalar.activation(dA[:], u[:], ACTF.Exp,
                                             bias=0.0, scale=t_negA[:, mc, n:n + 1])
                        nc.vector.memset(
                            dA.rearrange("p (b l) -> p b l", b=NB)[:, 1:NB, 0:1], 0.0)
                        dbx = scp.tile([P, T], F16, tag="dbx")
                        nc.gpsimd.tensor_mul(dbx[:], dtx[:], Bb[:, n8])
                        nc.vector.tensor_tensor_scan(dbx[:], dA[:], dbx[:], 0.0,
                                                     op0=ALU.mult, op1=ALU.add)
                        nc.vector.tensor_mul(dbx[:], dbx[:], Cb[:, n8])
                        if n8 % 2 == 0:
                            ch_prev = dbx
                        else:
                            nc.vector.tensor_add(pr[:, n8 // 2], ch_prev[:], dbx[:])
                    if blk == 0:
                        nc.vector.tensor_add(y4[:, mc], pr[:, 0], pr[:, 1])
                    else:
                        nc.vector.tensor_add(pr[:, 0], pr[:, 0], pr[:, 1])
                        nc.vector.tensor_add(y4[:, mc], y4[:, mc], pr[:, 0])
            # gate + out_proj (two oc-pair passes, ym via DRAM)
            ym_dram = dr.tile([P, CD, T], F16, name="ymdram")
            t_t = ap.tile([P, CO, NB, L], F16, tag="vt")   # reuse v_t slot
            for ph in range(2):
                ps_op = [popj.tile([P, 512], F32, tag=f"opj{i}", name=f"op{ph}_{i}")
                         for i in range(4)]
                for mc in range(CD):
                    ym = sl6.tile([P, T], F16, tag="s2k", name="ym")
                    if ph == 0:
                        xcsl = sl6.tile([P, T], F16, tag="s2k", name="xcsl")
                        nc.sync.dma_start(xcsl[:], xcs_dram[:, mc])
                        y2 = sl6.tile([P, T], F16, tag="s2k", name="y2")
                        nc.vector.scalar_tensor_tensor(
                            y2[:], xcsl[:], t_dskip[:, mc:mc + 1],
                            y4[:, mc], op0=ALU.mult, op1=ALU.add)
                        zs = sl6.tile([P, T], F16, tag="s2k", name="zsl")
                        nc.sync.dma_start(zs[:], z_dram[:, mc])
                        nc.scalar.activation(zs[:], zs[:], ACTF.Silu, bias=0.0, scale=1.0)
                        nc.vector.tensor_mul(ym[:], y2[:], zs[:])
                        nc.sync.dma_start(ym_dram[:, mc], ym[:])
                    else:
                        nc.sync.dma_start(ym[:], ym_dram[:, mc])
                    for j in range(2):
                        oc = ph * 2 + j
                        for h in range(2):
                            nc.tensor.matmul(ps_op[j * 2 + h][:],
                                             t_wout[:, mc, oc * P:(oc + 1) * P],
                                             ym[:, 512 * h:512 * (h + 1)],
                                             start=(mc == 0), stop=(mc == CD - 1))
                for j in range(2):
                    oc = ph * 2 + j
                    for h in range(2):
                        nc.vector.tensor_add(
                            t_t[:, oc, 2 * h:2 * h + 2, :],
                            out_t[:, oc, 2 * h:2 * h + 2, :],
                            ps_op[j * 2 + h].rearrange("p (b l) -> p b l", b=2))

            # ============ P7: LN2 + MLP (mc-blocked fc1/fc2) ============
            xh2 = ap.tile([P, CO, T], F16, tag="xh16")
            layernorm_16(t_t, xh2, "b")
            t_fc1 = load(wp, wfc1_16, tag="bigw", name="fc1w")
            h_dram = dr.tile([P, CM, T], F16, name="hdram")
            for mc in range(CM):
                hcs = sl6.tile([P, T], F16, tag="s2k", name="hcs")
                for h in range(2):
                    pt = pmm.tile([P, 512], F32, tag="mm", name="ptf")
                    for kc in range(CO):
                        nc.tensor.matmul(pt[:], t_fc1[:, kc, mc * P:(mc + 1) * P],
                                         xh2[:, kc, 512 * h:512 * (h + 1)],
                                         start=(kc == 0), stop=(kc == CO - 1))
                    nc.scalar.activation(hcs[:, 512 * h:512 * (h + 1)], pt[:],
                                         ACTF.Gelu, bias=t_bfc1[:, mc:mc + 1],
                                         scale=1.0)
                nc.sync.dma_start(h_dram[:, mc], hcs[:])
            t_fc2 = load(wp, wfc2_16, tag="bigw", name="fc2w")
            t2_t = p16.tile([P, CO, NB, L], F16, tag="p16", name="t2t")
            for ph in range(2):
                ps_f2 = [popj.tile([P, 512], F32, tag=f"opj{i}", name=f"f2{ph}_{i}")
                         for i in range(4)]
                for mc in range(CM):
                    hcs = sl6.tile([P, T], F16, tag="s2k", name="hcsr")
                    nc.sync.dma_start(hcs[:], h_dram[:, mc])
                    for j in range(2):
                        oc = ph * 2 + j
                        for h in range(2):
                            nc.tensor.matmul(ps_f2[j * 2 + h][:],
                                             t_fc2[:, mc, oc * P:(oc + 1) * P],
                                             hcs[:, 512 * h:512 * (h + 1)],
                                             start=(mc == 0), stop=(mc == CM - 1))
                for j in range(2):
                    oc = ph * 2 + j
                    for h in range(2):
                        fb = sl.tile([P, 2, L], F32, tag="fcb")
                        nc.scalar.activation(
                            fb[:], ps_f2[j * 2 + h].rearrange("p (b l) -> p b l", b=2),
                            ACTF.Identity, bias=t_fc2b[:, oc:oc + 1], scale=1.0)
                        nc.vector.tensor_add(t2_t[:, oc, 2 * h:2 * h + 2, :],
                                             t_t[:, oc, 2 * h:2 * h + 2, :], fb[:])

            # ============ P8: residual 1x1 conv + final ============
            t_resw = load(wp, res_wT, tag="bigw", name="resw")
            t_xpad2 = load(wp, x_pad, tag="bigw", name="xpad2")
            for oc in range(CO):
                for h in range(2):
                    pr2 = pmm.tile([P, 512], F32, tag="mm", name="prr")
                    for ic in range(CI):
                        nc.tensor.matmul(
                            pr2.rearrange("p (a b) -> p a b", a=2),
                            t_resw[:, ic, oc * P:(oc + 1) * P],
                            t_xpad2[:, ic, 2 * h:2 * h + 2, 1:1 + L],
                            start=(ic == 0), stop=(ic == CI - 1))
                    rv = sl.tile([P, 2, L], F32, tag="fcb")
                    nc.scalar.activation(rv[:], pr2.rearrange("p (b l) -> p b l", b=2),
                                         ACTF.Identity, bias=t_resb[:, oc:oc + 1], scale=1.0)
                    of = sl.tile([P, 2, L], F32, tag="ofin")
                    nc.vector.tensor_add(of[:], t2_t[:, oc, 2 * h:2 * h + 2, :], rv[:])
                    nc.sync.dma_start(out_d[:, oc, 2 * h:2 * h + 2, :], of[:])

    nc.compile()
    return nc


_prog = None
last_results = None


def kernel(**inputs):
    global _prog, last_results
    f = {k: np.asarray(v) for k, v in inputs.items()}

    conv_wT = _chunks(np.ascontiguousarray(f["conv_w"].transpose(1, 2, 0)))
    res_wT = _chunks(np.ascontiguousarray(f["res_w"].T))
    film_w16 = _chunks(f["film_w"]).astype(np.float16)
    win16 = _chunks(f["norm1_w"][:, None] * f["in_proj_w"]).astype(np.float16)
    bin_full = f["norm1_b"] @ f["in_proj_w"]
    xproj16 = _chunks(f["x_proj_w"]).astype(np.float16)
    dtproj16 = f["dt_proj_w"].astype(np.float16)
    negA = _chunks(np.exp(f["A_log"].astype(np.float64)).astype(np.float32))
    wfc1_16 = _chunks(f["norm2_w"][:, None] * f["fc1_w"]).astype(np.float16)
    bfc1_full = f["fc1_b"] + f["norm2_b"] @ f["fc1_w"]
    wfc2_16 = _chunks(f["fc2_w"]).astype(np.float16)
    wout16 = _chunks(f["out_proj_w"]).astype(np.float16)
    gmask = np.zeros((P, 2), np.float16)
    gmask[:64, 0] = 1.0
    gmask[64:, 1] = 1.0
    gexpand = np.zeros((2, P), np.float32)
    gexpand[0, :64] = 1.0
    gexpand[1, 64:] = 1.0

    shared = dict(
        conv_wT=conv_wT, res_wT=res_wT,
        gn_w=_cols(f["gn_w"]), gn_wneg=_cols(-f["gn_w"]), gn_b=_cols(f["gn_b"]),
        conv_b=_cols(f["conv_b"]), res_b=_cols(f["res_b"]),
        film_w16=film_w16, film_b=_cols(f["film_b"]),
        win16=win16, bin_c=_cols(bin_full),
        xproj16=xproj16, dtproj16=dtproj16,
        dtb_neg=_cols(-f["dt_proj_b"]), negA=negA,
        dconv_wc=_chunks(f["dconv_w"]), dconv_bc=_cols(f["dconv_b"]),
        dskip_c=_cols(f["Dskip"]), wout16=wout16,
        wfc1_16=wfc1_16, bfc1_c=_cols(bfc1_full),
        wfc2_16=wfc2_16, fc2_bc=_cols(f["fc2_b"]),
        gmask=gmask, gexpand=gexpand,
        ones_c=np.ones((P, 1), np.float16),
    )

    in_maps = []
    for c in range(NCORES):
        xb = f["x"][NB * c:NB * (c + 1)]
        xp = np.zeros((CIN, NB, L + K), np.float32)
        xp[:, :, 1:1 + L] = xb.transpose(1, 0, 2)
        in_maps.append(dict(shared, x_pad=_chunks(xp),
                            cond_cf=_chunks(np.ascontiguousarray(
                                f["cond"][NB * c:NB * (c + 1)].T))))

    if _prog is None:
        _prog = build_program()
    import os as _os
    _trace = bool(_os.environ.get('KERNEL_TRACE'))
    res = run_bass_kernel_spmd(_prog, in_maps, core_ids=list(range(NCORES)),
                               trace=_trace)
    last_results = res

    outs = []
    for c in range(NCORES):
        o = res.results[c]["out"]
        o = o.transpose(1, 0, 2, 3).reshape(COUT, NB, L).transpose(1, 0, 2)
        outs.append(o)
    return np.concatenate(outs, axis=0).astype(np.float32)


# revision 17
# speedup vs baseline: 1.2926x; 1.2926x over previous
import sys
sys.path.insert(0, '/opt/trn_rl_repo')
import numpy as np

import concourse.bacc as bacc
import concourse.tile as tile
import concourse.mybir as mybir
from concourse.bass_utils import run_bass_kernel_spmd

F32 = mybir.dt.float32
F32R = mybir.dt.float32r
F16 = mybir.dt.float16
ALU = mybir.AluOpType
ACTF = mybir.ActivationFunctionType

# problem dims (hardcoded)
B, CIN, COUT, L, COND = 32, 256, 512, 256, 512
DSTATE, DCONV = 16, 4
DIN = 1024
DTR = 32
MLPH = 2048
NG = 8
K = 3
NCORES = 8
NB = B // NCORES          # 4 batches per core
T = NB * L                # 1024 tokens per core
P = 128
CI = CIN // P             # 2 input-channel chunks
CO = COUT // P            # 4 out-channel chunks
CD = DIN // P             # 8 d_inner chunks
CM = MLPH // P            # 16 mlp chunks
GSZ = (COUT // NG) * L    # group-norm group size = 64*256
NBLK = 4                  # n-state blocks for B/C broadcast
NSB = DSTATE // NBLK      # 8 states per block


def _chunks(w):
    kt = w.shape[0]
    return np.ascontiguousarray(
        w.reshape(kt // P, P, *w.shape[1:]).transpose(1, 0, *range(2, w.ndim + 1)))


def _cols(v):
    return np.ascontiguousarray(v.reshape(-1, P).T)


def build_program():
    nc = bacc.Bacc(trn_type="TRN2")

    def din(name, shape, dt):
        return nc.dram_tensor(name, list(shape), dt, kind="ExternalInput")

    x_pad = din("x_pad", [P, CI, NB, L + K], F32R)
    cond_cf = din("cond_cf", [P, COND // P, NB], F32)
    conv_wT = din("conv_wT", [P, CI, K, COUT], F32R)
    res_wT = din("res_wT", [P, CI, COUT], F32R)
    gn_w = din("gn_w", [P, CO], F32)
    gn_wneg = din("gn_wneg", [P, CO], F32)
    gn_b = din("gn_b", [P, CO], F32)
    conv_b = din("conv_b", [P, CO], F32)
    res_b = din("res_b", [P, CO], F32)
    film_w16 = din("film_w16", [P, COND // P, 2 * COUT], F16)
    film_b = din("film_b", [P, 2 * COUT // P], F32)
    win16 = din("win16", [P, CO, 2 * DIN], F16)
    bin_c = din("bin_c", [P, 2 * DIN // P], F32)
    xproj16 = din("xproj16", [P, CD, DTR + 2 * DSTATE], F16)
    dtproj16 = din("dtproj16", [DTR, DIN], F16)
    dtb_neg = din("dtb_neg", [P, CD], F32)
    negA = din("negA", [P, CD, DSTATE], F32)
    dconv_wc = din("dconv_wc", [P, CD, DCONV], F32)
    dconv_bc = din("dconv_bc", [P, CD], F32)
    dskip_c = din("dskip_c", [P, CD], F32)
    wout16 = din("wout16", [P, CD, COUT], F16)
    wfc1_16 = din("wfc1_16", [P, CO, MLPH], F16)
    bfc1_c = din("bfc1_c", [P, CM], F32)
    wfc2_16 = din("wfc2_16", [P, CM, COUT], F16)
    fc2_bc = din("fc2_bc", [P, CO], F32)
    gmask = din("gmask", [P, 2], F16)
    gexpand = din("gexpand", [2, P], F32)
    ones_c = din("ones_c", [P, 1], F16)

    out_d = nc.dram_tensor("out", [P, CO, NB, L], F32, kind="ExternalOutput")

    with tile.TileContext(nc) as tc, nc.allow_low_precision("fp16 state by design"):
        with tc.tile_pool(name="wp", bufs=2) as wp, \
             tc.tile_pool(name="cst", bufs=1) as cst, \
             tc.tile_pool(name="act", bufs=1) as ap, \
             tc.tile_pool(name="p16", bufs=2) as p16, \
             tc.tile_pool(name="b16", bufs=3) as b16, \
             tc.tile_pool(name="row", bufs=4) as rowp, \
             tc.tile_pool(name="sl", bufs=2) as sl, \
             tc.tile_pool(name="sl6", bufs=8) as sl6, \
             tc.tile_pool(name="sl1", bufs=1) as sl1, \
             tc.tile_pool(name="sc", bufs=3) as scp, \
             tc.tile_pool(name="dram", bufs=1, space="DRAM") as dr, \
             tc.tile_pool(name="mm", bufs=2, space="PSUM") as pmm, \
             tc.tile_pool(name="pst", bufs=2, space="PSUM") as pst, \
             tc.tile_pool(name="opj", bufs=1, space="PSUM") as popj:

            def load(pool, dram, tag=None, name=None):
                t = pool.tile(list(dram.shape), dram.dtype, tag=tag,
                              name=name or "t_" + dram.name)
                nc.sync.dma_start(t[:], dram[:])
                return t

            t_gnw = load(cst, gn_w)
            t_gnwn = load(cst, gn_wneg)
            t_gnb = load(cst, gn_b)
            t_convb = load(cst, conv_b)
            t_resb = load(cst, res_b)
            t_filmb = load(cst, film_b)
            t_binc = load(cst, bin_c)
            t_xproj = load(cst, xproj16)
            t_dtproj = load(cst, dtproj16)
            t_dtbn = load(cst, dtb_neg)
            t_negA = load(cst, negA)
            t_dconvw = load(cst, dconv_wc)
            t_dconvb = load(cst, dconv_bc)
            t_dskip = load(cst, dskip_c)
            t_bfc1 = load(cst, bfc1_c)
            t_fc2b = load(cst, fc2_bc)
            t_gmask = load(cst, gmask)
            t_gexp = load(cst, gexpand)
            t_ones = load(cst, ones_c)
            t_cond = load(cst, cond_cf)
            eps2 = cst.tile([2, 1], F32)
            nc.vector.memset(eps2[:], 1e-5)
            eps1 = cst.tile([1, 1], F32)
            nc.vector.memset(eps1[:], 1e-5)

            # ============ P1: conv(k=3) ============
            t_xpad = load(wp, x_pad, tag="bigw", name="xpad1")
            t_convw = load(wp, conv_wT, tag="bigw", name="convw")
            conv_out = p16.tile([P, CO, NB, L], F16, tag="p16", name="convout")
            for oc in range(CO):
                for bp in range(NB // 2):
                    pt = pmm.tile([P, 2, L], F32, tag="mm", name="ptc")
                    first = True
                    for k in range(K):
                        for ic in range(CI):
                            nc.tensor.matmul(
                                pt[:], t_convw[:, ic, k, oc * P:(oc + 1) * P],
                                t_xpad[:, ic, 2 * bp:2 * bp + 2, k:k + L],
                                start=first, stop=(k == K - 1 and ic == CI - 1))
                            first = False
                    nc.scalar.activation(
                        conv_out[:, oc, 2 * bp:2 * bp + 2, :], pt[:],
                        ACTF.Identity, bias=t_convb[:, oc:oc + 1], scale=1.0)

            # ============ P2: GroupNorm stats ============
            s1 = ap.tile([2, CO, NB, 1], F32, tag="s1")
            s2 = ap.tile([2, CO, NB, 1], F32, tag="s2")
            for oc in range(CO):
                sq = sl.tile([P, NB, L], F16, tag="sq")
                nc.scalar.activation(sq[:], conv_out[:, oc], ACTF.Square,
                                     bias=0.0, scale=1.0)
                for h in range(2):
                    p1 = pmm.tile([2, 2, L], F32, tag="mm", name="pg1")
                    nc.tensor.matmul(p1.rearrange("p a b -> p (a b)"), t_gmask[:],
                                     conv_out[:, oc, 2 * h:2 * h + 2, :].rearrange(
                                         "p a b -> p (a b)"),
                                     start=True, stop=True)
                    nc.vector.tensor_reduce(s1[:, oc, 2 * h:2 * h + 2, :], p1[:],
                                            axis=mybir.AxisListType.X, op=ALU.add)
                    p2 = pmm.tile([2, 2, L], F32, tag="mm", name="pg2")
                    nc.tensor.matmul(p2.rearrange("p a b -> p (a b)"), t_gmask[:],
                                     sq[:, 2 * h:2 * h + 2, :].rearrange(
                                         "p a b -> p (a b)"),
                                     start=True, stop=True)
                    nc.vector.tensor_reduce(s2[:, oc, 2 * h:2 * h + 2, :], p2[:],
                                            axis=mybir.AxisListType.X, op=ALU.add)
            mean_g = ap.tile([2, CO, NB, 1], F32, tag="meang")
            nc.vector.tensor_scalar_mul(mean_g[:], s1[:], 1.0 / GSZ)
            msq = ap.tile([2, CO, NB, 1], F32, tag="msq")
            nc.vector.tensor_mul(msq[:], mean_g[:], mean_g[:])
            var_g = ap.tile([2, CO, NB, 1], F32, tag="varg")
            nc.vector.scalar_tensor_tensor(var_g[:], s2[:], 1.0 / GSZ, msq[:],
                                           op0=ALU.mult, op1=ALU.subtract)
            rstd_g = ap.tile([2, CO, NB, 1], F32, tag="rstdg")
            nc.scalar.activation(rstd_g[:], var_g[:], ACTF.Ln, bias=eps2[:], scale=1.0)
            nc.scalar.activation(rstd_g[:], rstd_g[:], ACTF.Exp, bias=0.0, scale=-0.5)
            mr_g = ap.tile([2, CO, NB, 1], F32, tag="mrg")
            nc.vector.tensor_mul(mr_g[:], mean_g[:], rstd_g[:])
            exp_in = ap.tile([2, CO, 2, NB], F32, tag="expin")
            nc.vector.tensor_copy(exp_in[:, :, 0, :], rstd_g[:, :, :, 0])
            nc.vector.tensor_copy(exp_in[:, :, 1, :], mr_g[:, :, :, 0])
            exp_t = ap.tile([P, CO, 2, NB], F32, tag="expt")
            for oc in range(CO):
                pe = pmm.tile([P, 2 * NB], F32, tag="mm", name="pge")
                nc.tensor.matmul(pe[:], t_gexp[:],
                                 exp_in[:, oc].rearrange("p a b -> p (a b)"),
                                 start=True, stop=True)
                nc.scalar.copy(exp_t[:, oc], pe.rearrange("p (a b) -> p a b", a=2))
            gnA = ap.tile([P, CO, NB], F32, tag="gnA")
            gnB = ap.tile([P, CO, NB], F32, tag="gnB")
            for oc in range(CO):
                nc.vector.tensor_scalar_mul(gnA[:, oc], exp_t[:, oc, 0], t_gnw[:, oc:oc + 1])
                nc.vector.scalar_tensor_tensor(
                    gnB[:, oc], exp_t[:, oc, 1], t_gnwn[:, oc:oc + 1],
                    t_gnb[:, oc:oc + 1].to_broadcast([P, NB]),
                    op0=ALU.mult, op1=ALU.add)

            # ============ P3: GN apply + mish + FiLM ============
            v_t = ap.tile([P, CO, NB, L], F16, tag="vt")
            for oc in range(CO):
                for b in range(NB):
                    nc.vector.scalar_tensor_tensor(
                        v_t[:, oc, b], conv_out[:, oc, b], gnA[:, oc, b:b + 1],
                        gnB[:, oc, b:b + 1].to_broadcast([P, L]),
                        op0=ALU.mult, op1=ALU.add)
            msh = [rowp.tile([P, NB, L], F16, tag="row", name=f"msh{oc}")
                   for oc in range(CO)]
            csg = ap.tile([P, COND // P, NB], F32, tag="csg")
            for oc in range(CO):
                nc.scalar.activation(msh[oc][:], v_t[:, oc], ACTF.Sigmoid,
                                     bias=0.0, scale=-1.0)
            nc.scalar.activation(csg[:], t_cond[:], ACTF.Sigmoid, bias=0.0, scale=-1.0)
            for oc in range(CO):
                nc.scalar.activation(msh[oc][:], msh[oc][:], ACTF.Ln, bias=0.0, scale=1.0)
            nc.scalar.activation(csg[:], csg[:], ACTF.Ln, bias=0.0, scale=1.0)
            for oc in range(CO):
                nc.scalar.activation(msh[oc][:], msh[oc][:], ACTF.Tanh, bias=0.0, scale=-1.0)
            nc.scalar.activation(csg[:], csg[:], ACTF.Tanh, bias=0.0, scale=-1.0)
            for oc in range(CO):
                nc.vector.tensor_mul(v_t[:, oc], v_t[:, oc], msh[oc][:])
            mish_c = ap.tile([P, COND // P, NB], F16, tag="mishc")
            nc.vector.tensor_mul(mish_c[:], t_cond[:], csg[:])
            t_filmw = load(wp, film_w16, tag="bigw", name="filmw")
            film_sc = ap.tile([P, 2 * COUT // P, NB], F32, tag="filmsc")
            for mc in range(2 * COUT // P):
                pf = pmm.tile([P, NB], F32, tag="mm", name="pfm")
                for kc in range(COND // P):
                    nc.tensor.matmul(pf[:], t_filmw[:, kc, mc * P:(mc + 1) * P],
                                     mish_c[:, kc], start=(kc == 0),
                                     stop=(kc == COND // P - 1))
                nc.scalar.activation(film_sc[:, mc], pf[:], ACTF.Identity,
                                     bias=t_filmb[:, mc:mc + 1], scale=1.0)
            out_t = ap.tile([P, CO, NB, L], F16, tag="outt")
            for oc in range(CO):
                for b in range(NB):
                    nc.vector.scalar_tensor_tensor(
                        out_t[:, oc, b], v_t[:, oc, b], film_sc[:, oc, b:b + 1],
                        film_sc[:, CO + oc, b:b + 1].to_broadcast([P, L]),
                        op0=ALU.mult, op1=ALU.add)

            # ============ layernorm helper (src f16 [P,CO,NB,L] -> dst f16 [P,CO,T]) ====
            def layernorm_16(src, xh, tag):
                stt1 = pst.tile([33, 512], F32, tag="st", name="st1" + tag)
                stt2 = pst.tile([33, 512], F32, tag="st", name="st2" + tag)
                ps1 = [stt1[0:1, :], stt1[32:33, :]]
                ps2 = [stt2[0:1, :], stt2[32:33, :]]
                for oc in range(CO):
                    sq = sl.tile([P, NB, L], F16, tag="sq")
                    nc.scalar.activation(sq[:], src[:, oc], ACTF.Square, bias=0.0, scale=1.0)
                    for h in range(2):
                        nc.tensor.matmul(ps1[h][:], t_ones[:],
                                         src[:, oc, 2 * h:2 * h + 2, :].rearrange(
                                             "p a b -> p (a b)"),
                                         start=(oc == 0), stop=(oc == CO - 1))
                        nc.tensor.matmul(ps2[h][:], t_ones[:],
                                         sq[:, 2 * h:2 * h + 2, :].rearrange(
                                             "p a b -> p (a b)"),
                                         start=(oc == 0), stop=(oc == CO - 1))
                s1r = rowp.tile([1, T], F32, tag="row", name="s1r" + tag)
                s2r = rowp.tile([1, T], F32, tag="row", name="s2r" + tag)
                for h in range(2):
                    nc.scalar.copy(s1r[:, 512 * h:512 * (h + 1)], ps1[h][:])
                    nc.scalar.copy(s2r[:, 512 * h:512 * (h + 1)], ps2[h][:])
                nc.vector.tensor_scalar_mul(s1r[:], s1r[:], 1.0 / COUT)   # mu
                musq = rowp.tile([1, T], F32, tag="row", name="musq" + tag)
                nc.vector.tensor_mul(musq[:], s1r[:], s1r[:])
                nc.vector.scalar_tensor_tensor(s2r[:], s2r[:], 1.0 / COUT, musq[:],
                                               op0=ALU.mult, op1=ALU.subtract)  # var
                nc.scalar.activation(s2r[:], s2r[:], ACTF.Ln, bias=eps1[:], scale=1.0)
                nc.scalar.activation(s2r[:], s2r[:], ACTF.Exp, bias=0.0, scale=-0.5)  # rs
                cc = rowp.tile([1, T], F32, tag="row", name="cc" + tag)
                nc.vector.scalar_tensor_tensor(cc[:], s1r[:], -1.0, s2r[:],
                                               op0=ALU.mult, op1=ALU.mult)  # -mu*rs
                rs16 = rowp.tile([1, T], F16, tag="row", name="rs16" + tag)
                nc.vector.tensor_copy(rs16[:], s2r[:])
                cc16 = rowp.tile([1, T], F16, tag="row", name="cc16" + tag)
                nc.vector.tensor_copy(cc16[:], cc[:])
                rs_b = sl1.tile([P, T], F16, tag="lnrsb")
                cc_b = sl1.tile([P, T], F16, tag="lnccb")
                nc.gpsimd.partition_broadcast(rs_b[:], rs16[:])
                nc.gpsimd.partition_broadcast(cc_b[:], cc16[:])
                for oc in range(CO):
                    tmp = sl1.tile([P, NB, L], F16, tag="lntmp")
                    nc.vector.tensor_mul(tmp[:], src[:, oc],
                                         rs_b.rearrange("p (b l) -> p b l", b=NB))
                    nc.vector.tensor_add(xh[:, oc],
                                         tmp.rearrange("p b l -> p (b l)"), cc_b[:])

            # ============ P4: LN1 + in_proj ============
            xh16 = ap.tile([P, CO, T], F16, tag="xh16")
            layernorm_16(out_t, xh16, "a")
            t_win = load(wp, win16, tag="bigw", name="winw")
            xc_pad = p16.tile([P, CD, NB, L + K], F16, tag="p16", name="xcpad")
            nc.vector.memset(xc_pad[:], 0.0)
            z_dram = dr.tile([P, CD, T], F16, name="zdram")
            for mc in range(2 * DIN // P):
                for h in range(2):
                    pt = pmm.tile([P, 512], F32, tag="mm", name="pti")
                    for kc in range(CO):
                        nc.tensor.matmul(pt[:], t_win[:, kc, mc * P:(mc + 1) * P],
                                         xh16[:, kc, 512 * h:512 * (h + 1)],
                                         start=(kc == 0), stop=(kc == CO - 1))
                    if mc < CD:
                        nc.scalar.activation(
                            xc_pad[:, mc, 2 * h:2 * h + 2, K:K + L],
                            pt.rearrange("p (b l) -> p b l", b=2),
                            ACTF.Identity, bias=t_binc[:, mc:mc + 1], scale=1.0)
                    else:
                        zz = sl6.tile([P, 512], F16, tag="s2k", name="zz")
                        nc.scalar.activation(zz[:], pt[:], ACTF.Identity,
                                             bias=t_binc[:, mc:mc + 1], scale=1.0)
                        nc.sync.dma_start(z_dram[:, mc - CD, 512 * h:512 * (h + 1)], zz[:])

            # ============ P5: dw-conv + silu + x_proj + dt ============
            xcs16 = p16.tile([P, CD, T], F16, tag="p16", name="xcs16")
            xcs_dram = dr.tile([P, CD, T], F16, name="xcsdram")
            for mc in range(CD):
                for bp in range(NB // 2):
                    acc = sl.tile([P, 2, L], F16, tag="dwacc")
                    nc.vector.tensor_scalar_mul(
                        acc[:], xc_pad[:, mc, 2 * bp:2 * bp + 2, 0:L],
                        t_dconvw[:, mc, 0:1])
                    for k in range(1, DCONV):
                        nc.vector.scalar_tensor_tensor(
                            acc[:], xc_pad[:, mc, 2 * bp:2 * bp + 2, k:k + L],
                            t_dconvw[:, mc, k:k + 1], acc[:],
                            op0=ALU.mult, op1=ALU.add)
                    nc.scalar.activation(
                        xcs16[:, mc, 512 * bp:512 * (bp + 1)],
                        acc.rearrange("p b l -> p (b l)"),
                        ACTF.Silu, bias=t_dconvb[:, mc:mc + 1], scale=1.0)
                nc.sync.dma_start(xcs_dram[:, mc], xcs16[:, mc])
            dbl = ap.tile([DTR + 2 * DSTATE, T], F16, tag="dbl")
            for h in range(2):
                pd = pmm.tile([DTR + 2 * DSTATE, 512], F32, tag="mm", name="pdx")
                for kc in range(CD):
                    nc.tensor.matmul(pd[:], t_xproj[:, kc],
                                     xcs16[:, kc, 512 * h:512 * (h + 1)],
                                     start=(kc == 0), stop=(kc == CD - 1))
                nc.scalar.copy(dbl[:, 512 * h:512 * (h + 1)], pd[:])
            sg_dram = dr.tile([P, CD, T], F16, name="sgdram")
            for mc in range(CD):
                us = sl6.tile([P, T], F16, tag="s2k", name="us")
                for h in range(2):
                    pt = pmm.tile([P, 512], F32, tag="mm", name="ptd")
                    nc.tensor.matmul(pt[:], t_dtproj[:, mc * P:(mc + 1) * P],
                                     dbl[0:DTR, 512 * h:512 * (h + 1)],
                                     start=True, stop=True)
                    nc.scalar.activation(us[:, 512 * h:512 * (h + 1)], pt[:],
                                         ACTF.Sigmoid, bias=t_dtbn[:, mc:mc + 1], scale=-1.0)
                nc.scalar.activation(us[:], us[:], ACTF.Ln, bias=0.0, scale=1.0)
                nc.sync.dma_start(sg_dram[:, mc], us[:])

            # ============ P6: selective scan (n-blocked) ============
            t_wout = load(wp, wout16, tag="bigw", name="woutw")
            y4 = p16.tile([P, CD, T], F16, tag="p16", name="y4")
            for blk in range(NBLK):
                Bb = b16.tile([P, NSB, T], F16, tag="b16", name=f"Bb{blk}")
                Cb = b16.tile([P, NSB, T], F16, tag="b16", name=f"Cb{blk}")
                for n8 in range(NSB):
                    n = blk * NSB + n8
                    stB = sl6.tile([1, T], F16, tag="s2k", name="stB")
                    nc.sync.dma_start(stB[:], dbl[DTR + n:DTR + n + 1, :])
                    nc.gpsimd.partition_broadcast(Bb[:, n8], stB[:])
                    stC = sl6.tile([1, T], F16, tag="s2k", name="stC")
                    nc.sync.dma_start(stC[:], dbl[DTR + DSTATE + n:DTR + DSTATE + n + 1, :])
                    nc.gpsimd.partition_broadcast(Cb[:, n8], stC[:])
                for mc in range(CD):
                    u = sl6.tile([P, T], F16, tag="s2k", name="u")
                    nc.sync.dma_start(u[:], sg_dram[:, mc])
                    dtxs = sl6.tile([P, T], F16, tag="s2k", name="dtxs")
                    nc.sync.dma_start(dtxs[:], xcs_dram[:, mc])
                    dtx = sl6.tile([P, T], F16, tag="s2k", name="dtx")
                    nc.vector.scalar_tensor_tensor(dtx[:], u[:], -1.0, dtxs[:],
                                                   op0=ALU.mult, op1=ALU.mult)
                    pr = sl1.tile([P, NSB // 2, T], F16, tag="pr")
                    ch_prev = None
                    for n8 in range(NSB):
                        n = blk * NSB + n8
                        dA = scp.tile([P, T], F16, tag="dA")
                        nc.scalar.activation(dA[:], u[:], ACTF.Exp,
                                             bias=0.0, scale=t_negA[:, mc, n:n + 1])
                        nc.vector.memset(
                            dA.rearrange("p (b l) -> p b l", b=NB)[:, 1:NB, 0:1], 0.0)
                        dbx = scp.tile([P, T], F16, tag="dbx")
                        nc.vector.tensor_mul(dbx[:], dtx[:], Bb[:, n8])
                        nc.vector.tensor_tensor_scan(dbx[:], dA[:], dbx[:], 0.0,
                                                     op0=ALU.mult, op1=ALU.add)
                        nc.vector.tensor_mul(dbx[:], dbx[:], Cb[:, n8])
                        if n8 % 2 == 0:
                            ch_prev = dbx
                        else:
                            nc.vector.tensor_add(pr[:, n8 // 2], ch_prev[:], dbx[:])
                    if blk == 0:
                        nc.vector.tensor_add(y4[:, mc], pr[:, 0], pr[:, 1])
                    else:
                        nc.vector.tensor_add(pr[:, 0], pr[:, 0], pr[:, 1])
                        nc.vector.tensor_add(y4[:, mc], y4[:, mc], pr[:, 0])
            # gate + out_proj (two oc-pair passes, ym via DRAM)
            ym_dram = dr.tile([P, CD, T], F16, name="ymdram")
            t_t = ap.tile([P, CO, NB, L], F16, tag="vt")   # reuse v_t slot
            for ph in range(2):
                ps_op = [popj.tile([P, 512], F32, tag=f"opj{i}", name=f"op{ph}_{i}")
                         for i in range(4)]
                for mc in range(CD):
                    ym = sl6.tile([P, T], F16, tag="s2k", name="ym")
                    if ph == 0:
                        xcsl = sl6.tile([P, T], F16, tag="s2k", name="xcsl")
                        nc.sync.dma_start(xcsl[:], xcs_dram[:, mc])
                        y2 = sl6.tile([P, T], F16, tag="s2k", name="y2")
                        nc.vector.scalar_tensor_tensor(
                            y2[:], xcsl[:], t_dskip[:, mc:mc + 1],
                            y4[:, mc], op0=ALU.mult, op1=ALU.add)
                        zs = sl6.tile([P, T], F16, tag="s2k", name="zsl")
                        nc.sync.dma_start(zs[:], z_dram[:, mc])
                        nc.scalar.activation(zs[:], zs[:], ACTF.Silu, bias=0.0, scale=1.0)
                        nc.vector.tensor_mul(ym[:], y2[:], zs[:])
                        nc.sync.dma_start(ym_dram[:, mc], ym[:])
                    else:
                        nc.sync.dma_start(ym[:], ym_dram[:, mc])
                    for j in range(2):
                        oc = ph * 2 + j
                        for h in range(2):
                            nc.tensor.matmul(ps_op[j * 2 + h][:],
                                             t_wout[:, mc, oc * P:(oc + 1) * P],
                                             ym[:, 512 * h:512 * (h + 1)],
                                             start=(mc == 0), stop=(mc == CD - 1))
                for j in range(2):
                    oc = ph * 2 + j
                    for h in range(2):
                        nc.vector.tensor_add(
                            t_t[:, oc, 2 * h:2 * h + 2, :],
                            out_t[:, oc, 2 * h:2 * h + 2, :],
                            ps_op[j * 2 + h].rearrange("p (b l) -> p b l", b=2))

            # ============ P7: LN2 + MLP (mc-blocked fc1/fc2) ============
            xh2 = ap.tile([P, CO, T], F16, tag="xh16")
            layernorm_16(t_t, xh2, "b")
            t_fc1 = load(wp, wfc1_16, tag="bigw", name="fc1w")
            h_dram = dr.tile([P, CM, T], F16, name="hdram")
            for mc in range(CM):
                hcs = sl6.tile([P, T], F16, tag="s2k", name="hcs")
                for h in range(2):
                    pt = pmm.tile([P, 512], F32, tag="mm", name="ptf")
                    for kc in range(CO):
                        nc.tensor.matmul(pt[:], t_fc1[:, kc, mc * P:(mc + 1) * P],
                                         xh2[:, kc, 512 * h:512 * (h + 1)],
                                         start=(kc == 0), stop=(kc == CO - 1))
                    nc.scalar.activation(hcs[:, 512 * h:512 * (h + 1)], pt[:],
                                         ACTF.Gelu, bias=t_bfc1[:, mc:mc + 1],
                                         scale=1.0)
                nc.sync.dma_start(h_dram[:, mc], hcs[:])
            t_fc2 = load(wp, wfc2_16, tag="bigw", name="fc2w")
            t2_t = p16.tile([P, CO, NB, L], F16, tag="p16", name="t2t")
            for ph in range(2):
                ps_f2 = [popj.tile([P, 512], F32, tag=f"opj{i}", name=f"f2{ph}_{i}")
                         for i in range(4)]
                for mc in range(CM):
                    hcs = sl6.tile([P, T], F16, tag="s2k", name="hcsr")
                    nc.sync.dma_start(hcs[:], h_dram[:, mc])
                    for j in range(2):
                        oc = ph * 2 + j
                        for h in range(2):
                            nc.tensor.matmul(ps_f2[j * 2 + h][:],
                                             t_fc2[:, mc, oc * P:(oc + 1) * P],
                                             hcs[:, 512 * h:512 * (h + 1)],
                                             start=(mc == 0), stop=(mc == CM - 1))
                for j in range(2):
                    oc = ph * 2 + j
                    for h in range(2):
                        fb = sl.tile([P, 2, L], F32, tag="fcb")
                        nc.scalar.activation(
                            fb[:], ps_f2[j * 2 + h].rearrange("p (b l) -> p b l", b=2),
                            ACTF.Identity, bias=t_fc2b[:, oc:oc + 1], scale=1.0)
                        nc.vector.tensor_add(t2_t[:, oc, 2 * h:2 * h + 2, :],
                                             t_t[:, oc, 2 * h:2 * h + 2, :], fb[:])

            # ============ P8: residual 1x1 conv + final ============
            t_resw = load(wp, res_wT, tag="bigw", name="resw")
            t_xpad2 = load(wp, x_pad, tag="bigw", name="xpad2")
            for oc in range(CO):
                for h in range(2):
                    pr2 = pmm.tile([P, 512], F32, tag="mm", name="prr")
                    for ic in range(CI):
                        nc.tensor.matmul(
                            pr2.rearrange("p (a b) -> p a b", a=2),
                            t_resw[:, ic, oc * P:(oc + 1) * P],
                            t_xpad2[:, ic, 2 * h:2 * h + 2, 1:1 + L],
                            start=(ic == 0), stop=(ic == CI - 1))
                    rv = sl.tile([P, 2, L], F32, tag="fcb")
                    nc.scalar.activation(rv[:], pr2.rearrange("p (b l) -> p b l", b=2),
                                         ACTF.Identity, bias=t_resb[:, oc:oc + 1], scale=1.0)
                    of = sl.tile([P, 2, L], F32, tag="ofin")
                    nc.vector.tensor_add(of[:], t2_t[:, oc, 2 * h:2 * h + 2, :], rv[:])
                    nc.sync.dma_start(out_d[:, oc, 2 * h:2 * h + 2, :], of[:])

    nc.compile()
    return nc


_prog = None
last_results = None


def kernel(**inputs):
    global _prog, last_results
    f = {k: np.asarray(v) for k, v in inputs.items()}

    conv_wT = _chunks(np.ascontiguousarray(f["conv_w"].transpose(1, 2, 0)))
    res_wT = _chunks(np.ascontiguousarray(f["res_w"].T))
    film_w16 = _chunks(f["film_w"]).astype(np.float16)
    win16 = _chunks(f["norm1_w"][:, None] * f["in_proj_w"]).astype(np.float16)
    bin_full = f["norm1_b"] @ f["in_proj_w"]
    xproj16 = _chunks(f["x_proj_w"]).astype(np.float16)
    dtproj16 = f["dt_proj_w"].astype(np.float16)
    negA = _chunks(np.exp(f["A_log"].astype(np.float64)).astype(np.float32))
    wfc1_16 = _chunks(f["norm2_w"][:, None] * f["fc1_w"]).astype(np.float16)
    bfc1_full = f["fc1_b"] + f["norm2_b"] @ f["fc1_w"]
    wfc2_16 = _chunks(f["fc2_w"]).astype(np.float16)
    wout16 = _chunks(f["out_proj_w"]).astype(np.float16)
    gmask = np.zeros((P, 2), np.float16)
    gmask[:64, 0] = 1.0
    gmask[64:, 1] = 1.0
    gexpand = np.zeros((2, P), np.float32)
    gexpand[0, :64] = 1.0
    gexpand[1, 64:] = 1.0

    shared = dict(
        conv_wT=conv_wT, res_wT=res_wT,
        gn_w=_cols(f["gn_w"]), gn_wneg=_cols(-f["gn_w"]), gn_b=_cols(f["gn_b"]),
        conv_b=_cols(f["conv_b"]), res_b=_cols(f["res_b"]),
        film_w16=film_w16, film_b=_cols(f["film_b"]),
        win16=win16, bin_c=_cols(bin_full),
        xproj16=xproj16, dtproj16=dtproj16,
        dtb_neg=_cols(-f["dt_proj_b"]), negA=negA,
        dconv_wc=_chunks(f["dconv_w"]), dconv_bc=_cols(f["dconv_b"]),
        dskip_c=_cols(f["Dskip"]), wout16=wout16,
        wfc1_16=wfc1_16, bfc1_c=_cols(bfc1_full),
        wfc2_16=wfc2_16, fc2_bc=_cols(f["fc2_b"]),
        gmask=gmask, gexpand=gexpand,
        ones_c=np.ones((P, 1), np.float16),
    )

    in_maps = []
    for c in range(NCORES):
        xb = f["x"][NB * c:NB * (c + 1)]
        xp = np.zeros((CIN, NB, L + K), np.float32)
        xp[:, :, 1:1 + L] = xb.transpose(1, 0, 2)
        in_maps.append(dict(shared, x_pad=_chunks(xp),
                            cond_cf=_chunks(np.ascontiguousarray(
                                f["cond"][NB * c:NB * (c + 1)].T))))

    if _prog is None:
        _prog = build_program()
    import os as _os
    _trace = bool(_os.environ.get('KERNEL_TRACE'))
    res = run_bass_kernel_spmd(_prog, in_maps, core_ids=list(range(NCORES)),
                               trace=_trace)
    last_results = res

    outs = []
    for c in range(NCORES):
        o = res.results[c]["out"]
        o = o.transpose(1, 0, 2, 3).reshape(COUT, NB, L).transpose(1, 0, 2)
        outs.append(o)
    return np.concatenate(outs, axis=0).astype(np.float32)
